# revision 14
# baseline (speedup 1.0000x reference)
"""Trainium2 Bass kernel for nn_SelDimeNet (DimeNet-style interaction block).

Strategy (8 NeuronCores, SPMD):
  - Triplets are assigned to the core that owns their idx_ji edge
    (edge range partition: core m owns edges [m*EC, (m+1)*EC)).
  - Pass 1 (selection pass, triplets sorted by angle bucket): device gathers
    x rows by idx_kj (indirect DMA), computes z = (silu(x@Wkj+b) * rbf_e) @ sel_w[s]
    with the selection matrix constant over long runs, writes z rows to DRAM.
  - Pass 2 (segment pass, triplets packed so that each edge's triplet segment
    lives entirely inside one 128-slot tile): gathers z rows, computes the
    bilinear acc_b = z @ W[:,b,:].T, scales by sbf_t (per-partition scalars),
    segment-sums via a one-hot matmul, and scatter-writes per-edge msg rows.
  - Edge pass: h = silu(x@Wji+b) + msg, residual MLP stack, data-parallel
    over the core's own edge range.

Host-side work is limited to sharding/scheduling: bucket/segment sorting and
packing of index metadata, the tiny sbf @ lin_sbf_w projection ([T,42]@[42,8],
0.17% of total FLOPs), and the rbf row gather (24B rows are not worth an
indirect-DMA descriptor storm on device).
"""

import math
import os
from contextlib import ExitStack
from dataclasses import dataclass

import numpy as np

import concourse.bacc as bacc
import concourse.bass as bass
import concourse.tile as tile
from concourse import mybir
from concourse.bass import AP
from concourse.bass_utils import run_bass_kernel_spmd
from concourse.masks import make_identity
from concourse.tile import TileContext

F32 = mybir.dt.float32
I32 = mybir.dt.int32
AF = mybir.ActivationFunctionType
OP = mybir.AluOpType

PI_CONST = np.float32(3.141593)


@dataclass(frozen=True)
class Cfg:
    NC: int = 8          # cores
    E: int = 400000      # edges
    T: int = 1200000     # triplets
    H: int = 128         # hidden
    B: int = 8           # num_bilinear
    NR: int = 6          # num_radial
    NSR: int = 42        # NS*NR
    SC: int = 8          # selection buckets
    B_PAD: int = 20480   # pass-1 per-bucket padded size (multiple of 128)
    NT2: int = 1250      # pass-2 tile count
    ECHUNK: int = 500    # edge-pass chunk (<=512, mult of 4 blocks <=128)

    @property
    def EC(self):
        return self.E // self.NC

    @property
    def T1(self):
        return self.SC * self.B_PAD

    @property
    def NT1(self):
        return self.T1 // 128

    @property
    def T2(self):
        return self.NT2 * 128

    @property
    def NCHUNK(self):
        assert self.EC % self.ECHUNK == 0
        return self.EC // self.ECHUNK

    @property
    def SCRATCH(self):
        return self.EC  # scratch msg row for dummy scatter targets

    @property
    def MSG_ROWS(self):
        return self.EC + 128


FULL = Cfg()


# --------------------------------------------------------------------------
# Host-side sharding / scheduling
# --------------------------------------------------------------------------

def host_prep(inputs, cfg: Cfg):
    """Build per-core input maps (list of dicts keyed by DRAM tensor name)."""
    c = cfg
    x = np.ascontiguousarray(np.asarray(inputs["x"], dtype=np.float32))
    rbf = np.asarray(inputs["rbf"], dtype=np.float32)
    sbf = np.asarray(inputs["sbf"], dtype=np.float32)
    angle = np.asarray(inputs["angle"], dtype=np.float32)
    idx_kj = np.asarray(inputs["idx_kj"]).astype(np.int64)
    idx_ji = np.asarray(inputs["idx_ji"]).astype(np.int64)

    lin_rbf_w = np.asarray(inputs["lin_rbf_w"], np.float32)
    lin_sbf_w = np.asarray(inputs["lin_sbf_w"], np.float32)
    lin_kj_w = np.asarray(inputs["lin_kj_w"], np.float32)
    lin_kj_b = np.asarray(inputs["lin_kj_b"], np.float32)
    lin_ji_w = np.asarray(inputs["lin_ji_w"], np.float32)
    lin_ji_b = np.asarray(inputs["lin_ji_b"], np.float32)
    W = np.asarray(inputs["W"], np.float32)
    sel_w = np.asarray(inputs["sel_w"], np.float32)
    rb_w = np.asarray(inputs["rb_w"], np.float32)
    rb_b = np.asarray(inputs["rb_b"], np.float32)
    ra_w = np.asarray(inputs["ra_w"], np.float32)
    ra_b = np.asarray(inputs["ra_b"], np.float32)
    lin_w = np.asarray(inputs["lin_w"], np.float32)
    lin_b = np.asarray(inputs["lin_b"], np.float32)

    # selection bucket, matching the reference float path exactly
    sel = np.floor(angle / PI_CONST * np.float32(c.SC)).astype(np.int32)
    np.clip(sel, 0, c.SC - 1, out=sel)

    # tiny host projection: sbf_t = sbf @ lin_sbf_w  [T, B]
    sbf_t = (sbf @ lin_sbf_w).astype(np.float32)

    owner = (idx_ji // c.EC).astype(np.int32)

    # shared weight tensors (identical per core)
    w_bil = np.ascontiguousarray(np.transpose(W, (1, 2, 0)))  # [B, Hin, Hout]
    shared = {
        "w_kj": np.ascontiguousarray(lin_kj_w),
        "b_kj": np.ascontiguousarray(lin_kj_b.reshape(c.H, 1)),
        "w_rbf": np.ascontiguousarray(lin_rbf_w),
        "w_sel": np.ascontiguousarray(sel_w),
        "w_bil": w_bil.astype(np.float32),
        "w_ji": np.ascontiguousarray(lin_ji_w),
        "b_ji": np.ascontiguousarray(lin_ji_b.reshape(c.H, 1)),
        "w_rb0": np.ascontiguousarray(rb_w[0, 0]),
        "b_rb0": np.ascontiguousarray(rb_b[0, 0].reshape(c.H, 1)),
        "w_rb1": np.ascontiguousarray(rb_w[0, 1]),
        "b_rb1": np.ascontiguousarray(rb_b[0, 1].reshape(c.H, 1)),
        "w_lin": np.ascontiguousarray(lin_w),
        "b_lin": np.ascontiguousarray(lin_b.reshape(c.H, 1)),
        "w_ra00": np.ascontiguousarray(ra_w[0, 0]),
        "b_ra00": np.ascontiguousarray(ra_b[0, 0].reshape(c.H, 1)),
        "w_ra01": np.ascontiguousarray(ra_w[0, 1]),
        "b_ra01": np.ascontiguousarray(ra_b[0, 1].reshape(c.H, 1)),
        "w_ra10": np.ascontiguousarray(ra_w[1, 0]),
        "b_ra10": np.ascontiguousarray(ra_b[1, 0].reshape(c.H, 1)),
        "w_ra11": np.ascontiguousarray(ra_w[1, 1]),
        "b_ra11": np.ascontiguousarray(ra_b[1, 1].reshape(c.H, 1)),
        "x_full": x,
    }

    rowmap = np.zeros(c.T, dtype=np.int64)  # triplet id -> ztab row
    in_maps = []
    for m in range(c.NC):
        tid = np.nonzero(owner == m)[0]
        tm = tid.size

        # ---------------- pass 1 layout (bucket-sorted, padded) ----------
        s_m = sel[tid]
        o1 = np.argsort(s_m, kind="stable")
        t1 = tid[o1]
        counts = np.bincount(s_m, minlength=c.SC)
        if counts.max() > c.B_PAD:
            raise RuntimeError(f"bucket overflow: {counts.max()} > {c.B_PAD}")
        pos1 = np.concatenate(
            [s * c.B_PAD + np.arange(counts[s], dtype=np.int64)
             for s in range(c.SC)]
        ) if tm else np.zeros(0, np.int64)

        idx_kj1 = np.zeros((c.T1, 1), dtype=np.int32)
        idx_kj1[pos1, 0] = idx_kj[t1].astype(np.int32)
        rbf1 = np.zeros((c.T1, c.NR), dtype=np.float32)
        rbf1[pos1] = rbf[idx_kj[t1]]
        rbf1T = np.ascontiguousarray(rbf1.T)

        rowmap[t1] = pos1

        # ---------------- pass 2 layout (segment-packed tiles) -----------
        o2 = np.argsort(idx_ji[tid], kind="stable")
        t2 = tid[o2]
        e2 = (idx_ji[t2] - m * c.EC).astype(np.int64)
        edges_u, seg_start, seg_cnt = np.unique(
            e2, return_index=True, return_counts=True)
        nseg = edges_u.size

        perm21 = np.zeros((c.T2, 1), dtype=np.int32)
        sbf2 = np.zeros((c.T2, c.B), dtype=np.float32)
        idxloc = np.full((c.T2, 1), 127.0, dtype=np.float32)
        etgt = np.full((c.T2, 1), c.SCRATCH, dtype=np.int32)

        tile_i = 0
        fill = 0
        rank = 0
        for si in range(nseg):
            cnt = int(seg_cnt[si])
            if cnt > 128:
                raise RuntimeError(f"segment larger than tile: {cnt}")
            if fill + cnt > 128:
                tile_i += 1
                fill = 0
                rank = 0
            if tile_i >= c.NT2:
                raise RuntimeError("pass-2 tile overflow")
            base = tile_i * 128
            trips = t2[seg_start[si]: seg_start[si] + cnt]
            sl = slice(base + fill, base + fill + cnt)
            perm21[sl, 0] = rowmap[trips].astype(np.int32)
            sbf2[sl] = sbf_t[trips]
            idxloc[sl, 0] = float(rank)
            etgt[base + rank, 0] = int(edges_u[si])
            fill += cnt
            rank += 1

        in_map = dict(shared)
        in_map.update(
            x_own=np.ascontiguousarray(x[m * c.EC:(m + 1) * c.EC]),
            idx_kj1=idx_kj1,
            rbf1T=rbf1T,
            perm21=perm21,
            sbf2=sbf2,
            idxloc2=idxloc,
            etgt2=etgt,
        )
        in_maps.append(in_map)
    return in_maps


# --------------------------------------------------------------------------
# Device program
# --------------------------------------------------------------------------

def _decl_inputs(nc: bacc.Bacc, cfg: Cfg):
    c = cfg
    t = {}

    def inp(name, shape, dtype=F32):
        t[name] = nc.dram_tensor(name, list(shape), dtype,
                                 kind="ExternalInput").ap()

    inp("x_full", (c.E, c.H))
    inp("x_own", (c.EC, c.H))
    inp("idx_kj1", (c.T1, 1), I32)
    inp("rbf1T", (c.NR, c.T1))
    inp("perm21", (c.T2, 1), I32)
    inp("sbf2", (c.T2, c.B))
    inp("idxloc2", (c.T2, 1))
    inp("etgt2", (c.T2, 1), I32)
    inp("w_kj", (c.H, c.H)); inp("b_kj", (c.H, 1))
    inp("w_rbf", (c.NR, c.H))
    inp("w_sel", (c.SC, c.H, c.H))
    inp("w_bil", (c.B, c.H, c.H))
    inp("w_ji", (c.H, c.H)); inp("b_ji", (c.H, 1))
    inp("w_rb0", (c.H, c.H)); inp("b_rb0", (c.H, 1))
    inp("w_rb1", (c.H, c.H)); inp("b_rb1", (c.H, 1))
    inp("w_lin", (c.H, c.H)); inp("b_lin", (c.H, 1))
    inp("w_ra00", (c.H, c.H)); inp("b_ra00", (c.H, 1))
    inp("w_ra01", (c.H, c.H)); inp("b_ra01", (c.H, 1))
    inp("w_ra10", (c.H, c.H)); inp("b_ra10", (c.H, 1))
    inp("w_ra11", (c.H, c.H)); inp("b_ra11", (c.H, 1))

    t["out_h"] = nc.dram_tensor("out_h", [c.EC, c.H], F32,
                                kind="ExternalOutput").ap()
    t["ztab"] = nc.dram_tensor("ztab", [c.T1, c.H], F32).ap()
    t["msg"] = nc.dram_tensor("msg", [c.MSG_ROWS, c.H], F32).ap()
    return t


def _load_w(nc, pool, dram_ap, shape, tag):
    sb = pool.tile(list(shape), F32, tag=tag)
    nc.sync.dma_start(out=sb[:], in_=dram_ap)
    return sb


def _pass1(ctx: ExitStack, tc: TileContext, t, cfg: Cfg):
    c = cfg
    nc = tc.nc
    wpool = ctx.enter_context(tc.tile_pool(name="w1", bufs=1))
    sb = ctx.enter_context(tc.tile_pool(name="sb1", bufs=3))
    ps_a = ctx.enter_context(tc.tile_pool(name="ps1a", bufs=2, space="PSUM"))
    ps_b = ctx.enter_context(tc.tile_pool(name="ps1b", bufs=1, space="PSUM"))

    ident = wpool.tile([128, 128], F32)
    make_identity(nc, ident[:])
    w_kj = _load_w(nc, wpool, t["w_kj"][:, :], (c.H, c.H), "w_kj")
    b_kj = _load_w(nc, wpool, t["b_kj"][:, :], (c.H, 1), "b_kj")
    w_rbf = _load_w(nc, wpool, t["w_rbf"][:, :], (c.NR, c.H), "w_rbf")
    w_sel = []
    for s in range(c.SC):
        w_sel.append(_load_w(nc, wpool, t["w_sel"][s, :, :], (c.H, c.H),
                             f"w_sel{s}"))

    # zero the msg table (consumed by the edge pass; scatter skips empty edges)
    zblk = 128
    zeros = wpool.tile([128, zblk], F32)
    nc.vector.memset(zeros[:], 0.0)
    nrows = c.MSG_ROWS
    r = 0
    while r < nrows:
        n = min(zblk, nrows - r)
        nc.sync.dma_start(out=t["msg"][r:r + n, :], in_=zeros[:n, :])
        r += n

    tiles_per_bucket = c.B_PAD // 128
    for i in range(c.NT1):
        s = i // tiles_per_bucket
        r0 = i * 128

        idxt = sb.tile([128, 1], I32)
        nc.sync.dma_start(out=idxt[:], in_=t["idx_kj1"][r0:r0 + 128, :])
        xg = sb.tile([128, 128], F32)
        nc.gpsimd.indirect_dma_start(
            out=xg[:], out_offset=None,
            in_=t["x_full"][:, :],
            in_offset=bass.IndirectOffsetOnAxis(ap=idxt[:, :1], axis=0),
        )
        rbft = sb.tile([c.NR, 128], F32)
        nc.sync.dma_start(out=rbft[:], in_=t["rbf1T"][:, r0:r0 + 128])

        xgT_ps = ps_a.tile([128, 128], F32)
        nc.tensor.transpose(out=xgT_ps[:], in_=xg[:], identity=ident[:])
        xgT = sb.tile([128, 128], F32)
        nc.vector.tensor_copy(xgT[:], xgT_ps[:])

        y_ps = ps_b.tile([128, 128], F32)
        nc.tensor.matmul(out=y_ps[:], lhsT=w_kj[:], rhs=xgT[:],
                         start=True, stop=True)
        rbfe_ps = ps_b.tile([128, 128], F32)
        nc.tensor.matmul(out=rbfe_ps[:], lhsT=w_rbf[:], rhs=rbft[:],
                         start=True, stop=True)

        ysil = sb.tile([128, 128], F32)
        nc.scalar.activation(ysil[:], y_ps[:], AF.Silu, bias=b_kj[:, :1])
        z1 = sb.tile([128, 128], F32)
        nc.vector.tensor_tensor(out=z1[:], in0=ysil[:], in1=rbfe_ps[:],
                                op=OP.mult)

        zT_ps = ps_b.tile([128, 128], F32)
        nc.tensor.matmul(out=zT_ps[:], lhsT=w_sel[s][:], rhs=z1[:],
                         start=True, stop=True)
        zT = sb.tile([128, 128], F32)
        nc.scalar.copy(zT[:], zT_ps[:])

        z_ps = ps_a.tile([128, 128], F32)
        nc.tensor.transpose(out=z_ps[:], in_=zT[:], identity=ident[:])
        z = sb.tile([128, 128], F32)
        nc.vector.tensor_copy(z[:], z_ps[:])
        nc.sync.dma_start(out=t["ztab"][r0:r0 + 128, :], in_=z[:])


def _pass2(ctx: ExitStack, tc: TileContext, t, cfg: Cfg):
    c = cfg
    nc = tc.nc
    wpool = ctx.enter_context(tc.tile_pool(name="w2", bufs=1))
    sb = ctx.enter_context(tc.tile_pool(name="sb2", bufs=3))
    ps_t = ctx.enter_context(tc.tile_pool(name="ps2t", bufs=2, space="PSUM"))
    ps_y = ctx.enter_context(tc.tile_pool(name="ps2y", bufs=2, space="PSUM"))
    ps_m = ctx.enter_context(tc.tile_pool(name="ps2m", bufs=2, space="PSUM"))

    ident = wpool.tile([128, 128], F32)
    make_identity(nc, ident[:])
    iota_i = wpool.tile([128, 128], I32)
    nc.gpsimd.iota(iota_i[:], pattern=[[1, 128]], base=0, channel_multiplier=0)
    iota_f = wpool.tile([128, 128], F32)
    nc.vector.tensor_copy(iota_f[:], iota_i[:])

    w_bil = wpool.tile([128, c.B * 128], F32)
    for b in range(c.B):
        nc.sync.dma_start(out=w_bil[:, b * 128:(b + 1) * 128],
                          in_=t["w_bil"][b, :, :])

    for i in range(c.NT2):
        r0 = i * 128
        idxt = sb.tile([128, 1], I32)
        nc.sync.dma_start(out=idxt[:], in_=t["perm21"][r0:r0 + 128, :])
        zg = sb.tile([128, 128], F32)
        nc.gpsimd.indirect_dma_start(
            out=zg[:], out_offset=None,
            in_=t["ztab"][:, :],
            in_offset=bass.IndirectOffsetOnAxis(ap=idxt[:, :1], axis=0),
        )
        sbft = sb.tile([128, c.B], F32)
        nc.sync.dma_start(out=sbft[:], in_=t["sbf2"][r0:r0 + 128, :])
        ilocal = sb.tile([128, 1], F32)
        nc.sync.dma_start(out=ilocal[:], in_=t["idxloc2"][r0:r0 + 128, :])
        etg = sb.tile([128, 1], I32)
        nc.sync.dma_start(out=etg[:], in_=t["etgt2"][r0:r0 + 128, :])

        zgT_ps = ps_t.tile([128, 128], F32)
        nc.tensor.transpose(out=zgT_ps[:], in_=zg[:], identity=ident[:])
        zgT = sb.tile([128, 128], F32)
        nc.vector.tensor_copy(zgT[:], zgT_ps[:])

        ypack_ps = ps_y.tile([128, c.B * 128], F32)
        for half in range(2):
            nc.tensor.matmul(
                out=ypack_ps[:, half * 512:(half + 1) * 512],
                lhsT=zgT[:],
                rhs=w_bil[:, half * 512:(half + 1) * 512],
                start=True, stop=True)

        tmp = sb.tile([128, c.B * 128], F32)
        for b in range(c.B):
            blk = slice(b * 128, (b + 1) * 128)
            if b < 4:
                nc.scalar.activation(tmp[:, blk], ypack_ps[:, blk], AF.Copy,
                                     scale=sbft[:, b:b + 1])
            else:
                nc.vector.tensor_scalar(
                    out=tmp[:, blk], in0=ypack_ps[:, blk],
                    scalar1=sbft[:, b:b + 1], scalar2=None, op0=OP.mult)

        smat = sb.tile([128, 128], F32)
        nc.vector.tensor_scalar(out=smat[:], in0=iota_f[:],
                                scalar1=ilocal[:, :1], scalar2=None,
                                op0=OP.is_equal)

        msg_ps = ps_m.tile([128, 128], F32)
        for b in range(c.B):
            nc.tensor.matmul(out=msg_ps[:], lhsT=smat[:],
                             rhs=tmp[:, b * 128:(b + 1) * 128],
                             start=(b == 0), stop=(b == c.B - 1))
        msg_sb = sb.tile([128, 128], F32)
        nc.scalar.copy(msg_sb[:], msg_ps[:])
        nc.gpsimd.indirect_dma_start(
            out=t["msg"][:, :],
            out_offset=bass.IndirectOffsetOnAxis(ap=etg[:, :1], axis=0),
            in_=msg_sb[:], in_offset=None,
        )


def _edge_pass(ctx: ExitStack, tc: TileContext, t, cfg: Cfg):
    c = cfg
    nc = tc.nc
    ch = c.ECHUNK
    nblk = 4
    blk = ch // nblk
    assert blk * nblk == ch and blk <= 128

    wpool = ctx.enter_context(tc.tile_pool(name="w3", bufs=1))
    sb = ctx.enter_context(tc.tile_pool(name="sb3", bufs=3))
    ps_x = ctx.enter_context(tc.tile_pool(name="ps3x", bufs=2, space="PSUM"))
    ps_g = ctx.enter_context(tc.tile_pool(name="ps3g", bufs=2, space="PSUM"))
    ps_o = ctx.enter_context(tc.tile_pool(name="ps3o", bufs=2, space="PSUM"))

    ident = wpool.tile([128, 128], F32)
    make_identity(nc, ident[:])
    names = ["w_ji", "w_rb0", "w_rb1", "w_lin", "w_ra00", "w_ra01",
             "w_ra10", "w_ra11"]
    ws = {n: _load_w(nc, wpool, t[n][:, :], (c.H, c.H), n) for n in names}
    bs = {n: _load_w(nc, wpool, t["b" + n[1:]][:, :], (c.H, 1), "b" + n[1:])
          for n in names}

    def mm_silu(w_name, rhs_sb):
        mm_ps = ps_g.tile([128, ch], F32)
        nc.tensor.matmul(out=mm_ps[:], lhsT=ws[w_name][:], rhs=rhs_sb[:],
                         start=True, stop=True)
        out = sb.tile([128, ch], F32)
        nc.scalar.activation(out[:], mm_ps[:], AF.Silu,
                             bias=bs[w_name][:, :1])
        return out

    for ci in range(c.NCHUNK):
        e0 = ci * ch

        xr = sb.tile([blk, nblk, 128], F32)
        nc.sync.dma_start(
            out=xr[:],
            in_=t["x_own"][e0:e0 + ch, :].rearrange("(n p) d -> p n d", p=blk))
        mr = sb.tile([blk, nblk, 128], F32)
        nc.sync.dma_start(
            out=mr[:],
            in_=t["msg"][e0:e0 + ch, :].rearrange("(n p) d -> p n d", p=blk))

        xT_ps = ps_x.tile([128, ch], F32)
        mT_ps = ps_x.tile([128, ch], F32)
        for k in range(nblk):
            ksl = slice(k * blk, (k + 1) * blk)
            nc.tensor.transpose(out=xT_ps[:, ksl], in_=xr[:, k, :],
                                identity=ident[:blk, :blk])
            nc.tensor.transpose(out=mT_ps[:, ksl], in_=mr[:, k, :],
                                identity=ident[:blk, :blk])
        xT = sb.tile([128, ch], F32)
        nc.vector.tensor_copy(xT[:], xT_ps[:])
        mT = sb.tile([128, ch], F32)
        nc.scalar.copy(mT[:], mT_ps[:])

        xji = mm_silu("w_ji", xT)
        h = sb.tile([128, ch], F32)
        nc.vector.tensor_tensor(out=h[:], in0=xji[:], in1=mT[:], op=OP.add)

        # residual layer before skip
        u = mm_silu("w_rb0", h)
        v = mm_silu("w_rb1", u)
        h2 = sb.tile([128, ch], F32)
        nc.vector.tensor_tensor(out=h2[:], in0=h[:], in1=v[:], op=OP.add)

        # skip connection
        l = mm_silu("w_lin", h2)
        h3 = sb.tile([128, ch], F32)
        nc.vector.tensor_tensor(out=h3[:], in0=l[:], in1=xT[:], op=OP.add)

        # residual layers after skip
        u2 = mm_silu("w_ra00", h3)
        v2 = mm_silu("w_ra01", u2)
        h4 = sb.tile([128, ch], F32)
        nc.vector.tensor_tensor(out=h4[:], in0=h3[:], in1=v2[:], op=OP.add)

        u3 = mm_silu("w_ra10", h4)
        v3 = mm_silu("w_ra11", u3)
        h5 = sb.tile([128, ch], F32)
        nc.vector.tensor_tensor(out=h5[:], in0=h4[:], in1=v3[:], op=OP.add)

        oT_ps = ps_o.tile([blk, nblk, 128], F32)
        for k in range(nblk):
            nc.tensor.transpose(out=oT_ps[:, k, :],
                                in_=h5[:, k * blk:(k + 1) * blk],
                                identity=ident[:])
        orow = sb.tile([blk, nblk, 128], F32)
        nc.vector.tensor_copy(orow[:], oT_ps[:])
        nc.sync.dma_start(
            out=t["out_h"][e0:e0 + ch, :].rearrange("(n p) d -> p n d", p=blk),
            in_=orow[:])


def build_program(cfg: Cfg):
    nc = bacc.Bacc("TRN2", target_bir_lowering=False, debug=False,
                   enable_asserts=False, num_devices=cfg.NC)
    t = _decl_inputs(nc, cfg)
    with TileContext(nc) as tc:
        with ExitStack() as ctx:
            _pass1(ctx, tc, t, cfg)
    with TileContext(nc) as tc:
        with ExitStack() as ctx:
            _pass2(ctx, tc, t, cfg)
    with TileContext(nc) as tc:
        with ExitStack() as ctx:
            _edge_pass(ctx, tc, t, cfg)
    nc.compile()
    return nc


_PROGRAM_CACHE = {}


def get_program(cfg: Cfg):
    key = cfg
    if key not in _PROGRAM_CACHE:
        _PROGRAM_CACHE[key] = build_program(cfg)
    return _PROGRAM_CACHE[key]


def kernel(**inputs):
    cfg = kernel.cfg or FULL
    in_maps = host_prep(inputs, cfg)
    nc = get_program(cfg)
    res = run_bass_kernel_spmd(nc, in_maps, list(range(cfg.NC)))
    kernel.last_results = res
    outs = [res.results[m]["out_h"] for m in range(cfg.NC)]
    return np.concatenate(outs, axis=0)


kernel.cfg = None
kernel.last_results = None


# revision 23
# speedup vs baseline: 1.2677x; 1.2677x over previous
"""Trainium2 Bass kernel for nn_SelDimeNet (DimeNet-style interaction block).

Strategy (8 NeuronCores, SPMD):
  - Triplets are assigned to the core that owns their idx_ji edge
    (edge range partition: core m owns edges [m*EC, (m+1)*EC)).
  - Pass 1 (selection pass, triplets sorted by angle bucket): device gathers
    x rows by idx_kj (indirect DMA), computes z = (silu(x@Wkj+b) * rbf_e) @ sel_w[s]
    with the selection matrix constant over long runs, writes z rows to DRAM.
  - Pass 2 (segment pass, triplets packed so that each edge's triplet segment
    lives entirely inside one 128-slot tile): gathers z rows, computes the
    bilinear acc_b = z @ W[:,b,:].T, scales by sbf_t (per-partition scalars),
    segment-sums via a one-hot matmul, and scatter-writes per-edge msg rows.
  - Edge pass: h = silu(x@Wji+b) + msg, residual MLP stack, data-parallel
    over the core's own edge range.

Host-side work is limited to sharding/scheduling: bucket/segment sorting and
packing of index metadata, the tiny sbf @ lin_sbf_w projection ([T,42]@[42,8],
0.17% of total FLOPs), and the rbf row gather (24B rows are not worth an
indirect-DMA descriptor storm on device).
"""

import math
import os
from contextlib import ExitStack
from dataclasses import dataclass

import numpy as np

import concourse.bacc as bacc
import concourse.bass as bass
import concourse.tile as tile
from concourse import mybir
from concourse.bass import AP
from concourse.bass_utils import run_bass_kernel_spmd
from concourse.masks import make_identity
from concourse.tile import TileContext

F32 = mybir.dt.float32
I32 = mybir.dt.int32
AF = mybir.ActivationFunctionType
OP = mybir.AluOpType

PI_CONST = np.float32(3.141593)


@dataclass(frozen=True)
class Cfg:
    NC: int = 8          # cores
    E: int = 400000      # edges
    T: int = 1200000     # triplets
    H: int = 128         # hidden
    B: int = 8           # num_bilinear
    NR: int = 6          # num_radial
    NSR: int = 42        # NS*NR
    SC: int = 8          # selection buckets
    B_PAD: int = 20480   # pass-1 per-bucket padded size (multiple of 512)
    EPT: int = 40        # pass-2 edges per tile (fixed window)
    NT2S: int = 32       # pass-2 spill tiles
    ECHUNK: int = 500    # edge-pass chunk (<=512, mult of 4 blocks <=128)

    @property
    def EC(self):
        return self.E // self.NC

    @property
    def T1(self):
        return self.SC * self.B_PAD

    @property
    def NT1(self):
        return self.T1 // 128

    @property
    def NT2(self):
        assert self.EC % self.EPT == 0
        return self.EC // self.EPT   # main pass-2 tiles

    @property
    def T2(self):
        return self.NT2 * 128

    @property
    def T2S(self):
        return self.NT2S * 128

    @property
    def NCHUNK(self):
        assert self.EC % self.ECHUNK == 0
        return self.EC // self.ECHUNK

    @property
    def SCRATCH(self):
        return self.EC  # scratch msg row for dummy spill-scatter targets

    @property
    def MSG_ROWS(self):
        return self.EC + 128


FULL = Cfg()


# --------------------------------------------------------------------------
# Host-side sharding / scheduling
# --------------------------------------------------------------------------

def host_prep(inputs, cfg: Cfg):
    """Build per-core input maps (list of dicts keyed by DRAM tensor name)."""
    c = cfg
    x = np.ascontiguousarray(np.asarray(inputs["x"], dtype=np.float32))
    rbf = np.asarray(inputs["rbf"], dtype=np.float32)
    sbf = np.asarray(inputs["sbf"], dtype=np.float32)
    angle = np.asarray(inputs["angle"], dtype=np.float32)
    idx_kj = np.asarray(inputs["idx_kj"]).astype(np.int64)
    idx_ji = np.asarray(inputs["idx_ji"]).astype(np.int64)

    lin_rbf_w = np.asarray(inputs["lin_rbf_w"], np.float32)
    lin_sbf_w = np.asarray(inputs["lin_sbf_w"], np.float32)
    lin_kj_w = np.asarray(inputs["lin_kj_w"], np.float32)
    lin_kj_b = np.asarray(inputs["lin_kj_b"], np.float32)
    lin_ji_w = np.asarray(inputs["lin_ji_w"], np.float32)
    lin_ji_b = np.asarray(inputs["lin_ji_b"], np.float32)
    W = np.asarray(inputs["W"], np.float32)
    sel_w = np.asarray(inputs["sel_w"], np.float32)
    rb_w = np.asarray(inputs["rb_w"], np.float32)
    rb_b = np.asarray(inputs["rb_b"], np.float32)
    ra_w = np.asarray(inputs["ra_w"], np.float32)
    ra_b = np.asarray(inputs["ra_b"], np.float32)
    lin_w = np.asarray(inputs["lin_w"], np.float32)
    lin_b = np.asarray(inputs["lin_b"], np.float32)

    # selection bucket, matching the reference float path exactly
    sel = np.floor(angle / PI_CONST * np.float32(c.SC)).astype(np.int32)
    np.clip(sel, 0, c.SC - 1, out=sel)

    # tiny host projection: sbf_t = sbf @ lin_sbf_w  [T, B]
    sbf_t = (sbf @ lin_sbf_w).astype(np.float32)

    owner = (idx_ji // c.EC).astype(np.int32)

    # shared weight tensors (identical per core)
    w_bil = np.ascontiguousarray(np.transpose(W, (1, 2, 0)))  # [B, Hin, Hout]
    shared = {
        "w_kj": np.ascontiguousarray(lin_kj_w),
        "b_kj": np.ascontiguousarray(lin_kj_b.reshape(c.H, 1)),
        "w_rbf": np.ascontiguousarray(lin_rbf_w),
        "w_sel": np.ascontiguousarray(sel_w),
        "w_bil": w_bil.astype(np.float32),
        "w_ji": np.ascontiguousarray(lin_ji_w),
        "b_ji": np.ascontiguousarray(lin_ji_b.reshape(c.H, 1)),
        "w_rb0": np.ascontiguousarray(rb_w[0, 0]),
        "b_rb0": np.ascontiguousarray(rb_b[0, 0].reshape(c.H, 1)),
        "w_rb1": np.ascontiguousarray(rb_w[0, 1]),
        "b_rb1": np.ascontiguousarray(rb_b[0, 1].reshape(c.H, 1)),
        "w_lin": np.ascontiguousarray(lin_w),
        "b_lin": np.ascontiguousarray(lin_b.reshape(c.H, 1)),
        "w_ra00": np.ascontiguousarray(ra_w[0, 0]),
        "b_ra00": np.ascontiguousarray(ra_b[0, 0].reshape(c.H, 1)),
        "w_ra01": np.ascontiguousarray(ra_w[0, 1]),
        "b_ra01": np.ascontiguousarray(ra_b[0, 1].reshape(c.H, 1)),
        "w_ra10": np.ascontiguousarray(ra_w[1, 0]),
        "b_ra10": np.ascontiguousarray(ra_b[1, 0].reshape(c.H, 1)),
        "w_ra11": np.ascontiguousarray(ra_w[1, 1]),
        "b_ra11": np.ascontiguousarray(ra_b[1, 1].reshape(c.H, 1)),
        "x_full": x,
    }

    rowmap = np.zeros(c.T, dtype=np.int64)  # triplet id -> ztab row
    in_maps = []
    for m in range(c.NC):
        tid = np.nonzero(owner == m)[0]
        tm = tid.size

        # ------- pass 1 layout (bucket-sorted, idx-sorted within bucket) --
        s_m = sel[tid]
        o1 = np.lexsort((idx_kj[tid], s_m))
        t1 = tid[o1]
        counts = np.bincount(s_m, minlength=c.SC)
        if counts.max() > c.B_PAD:
            raise RuntimeError(f"bucket overflow: {counts.max()} > {c.B_PAD}")
        pos1 = np.concatenate(
            [s * c.B_PAD + np.arange(counts[s], dtype=np.int64)
             for s in range(c.SC)]
        ) if tm else np.zeros(0, np.int64)

        idx_kj1 = np.zeros(c.T1, dtype=np.int32)
        idx_kj1[pos1] = idx_kj[t1].astype(np.int32)
        rbf1 = np.zeros((c.T1, c.NR), dtype=np.float32)
        rbf1[pos1] = rbf[idx_kj[t1]]
        rbf1T = np.ascontiguousarray(rbf1.T)
        # grouped gather-index layout: [T1/4, 4], row g*128+p col k = slot p
        # of tile 4g+k
        idx1g = np.ascontiguousarray(
            idx_kj1.reshape(c.NT1 // 4, 4, 128).transpose(0, 2, 1)
        ).reshape(c.T1 // 4, 4)

        rowmap[t1] = pos1

        # ---- pass 2: fixed 40-edge windows + spill for overfull windows --
        o2 = np.argsort(idx_ji[tid], kind="stable")
        t2 = tid[o2]
        e2 = (idx_ji[t2] - m * c.EC).astype(np.int64)
        win = e2 // c.EPT
        # window boundaries in the sorted triplet list
        wstart = np.searchsorted(win, np.arange(c.NT2 + 1))

        perm2 = np.zeros(c.T2, dtype=np.int32)
        meta2 = np.zeros((c.T2, 1 + c.B), dtype=np.float32)
        meta2[:, 0] = 127.0
        spill_segs = []  # (edge_local, trips)

        for k in range(c.NT2):
            lo, hi = int(wstart[k]), int(wstart[k + 1])
            trips = t2[lo:hi]
            eloc = e2[lo:hi] - k * c.EPT
            if hi - lo > 128:
                # spill whole segments (largest first) until it fits
                eds, scnt = np.unique(eloc, return_counts=True)
                order = np.argsort(scnt)[::-1]
                keep = np.ones(eloc.size, bool)
                n = eloc.size
                for oi in order:
                    if n <= 128:
                        break
                    mseg = eloc == eds[oi]
                    spill_segs.append(
                        (int(eds[oi]) + k * c.EPT, trips[mseg]))
                    keep &= ~mseg
                    n -= int(mseg.sum())
                trips = trips[keep]
                eloc = eloc[keep]
            base = k * 128
            nn = trips.size
            perm2[base:base + nn] = rowmap[trips].astype(np.int32)
            meta2[base:base + nn, 0] = eloc.astype(np.float32)
            meta2[base:base + nn, 1:] = sbf_t[trips]
        perm2g = np.ascontiguousarray(
            perm2.reshape(c.NT2 // 2, 2, 128).transpose(0, 2, 1)
        ).reshape(c.T2 // 2, 2)

        # spill tiles (classic rank-based scatter tiles)
        sperm = np.zeros((c.T2S, 1), dtype=np.int32)
        smeta = np.zeros((c.T2S, 1 + c.B), dtype=np.float32)
        smeta[:, 0] = 127.0
        setgt = np.full((c.T2S, 1), c.SCRATCH, dtype=np.int32)
        ti, fill, rank = 0, 0, 0
        for eg, trips in spill_segs:
            cnt = trips.size
            if fill + cnt > 128:
                ti += 1
                fill, rank = 0, 0
            if ti >= c.NT2S:
                raise RuntimeError("spill overflow")
            base = ti * 128
            sl = slice(base + fill, base + fill + cnt)
            sperm[sl, 0] = rowmap[trips].astype(np.int32)
            smeta[sl, 0] = float(rank)
            smeta[sl, 1:] = sbf_t[trips]
            setgt[base + rank, 0] = eg
            fill += cnt
            rank += 1

        in_map = dict(shared)
        in_map.update(
            x_own=np.ascontiguousarray(x[m * c.EC:(m + 1) * c.EC]),
            idx1g=idx1g,
            rbf1T=rbf1T,
            perm2g=perm2g,
            meta2=meta2,
            sperm=sperm,
            smeta=smeta,
            setgt=setgt,
        )
        in_maps.append(in_map)
    return in_maps


# --------------------------------------------------------------------------
# Device program
# --------------------------------------------------------------------------

def _decl_inputs(nc: bacc.Bacc, cfg: Cfg):
    c = cfg
    t = {}

    def inp(name, shape, dtype=F32):
        t[name] = nc.dram_tensor(name, list(shape), dtype,
                                 kind="ExternalInput").ap()

    inp("x_full", (c.E, c.H))
    inp("x_own", (c.EC, c.H))
    inp("idx1g", (c.T1 // 4, 4), I32)
    inp("rbf1T", (c.NR, c.T1))
    inp("perm2g", (c.T2 // 2, 2), I32)
    inp("meta2", (c.T2, 1 + c.B))
    inp("sperm", (c.T2S, 1), I32)
    inp("smeta", (c.T2S, 1 + c.B))
    inp("setgt", (c.T2S, 1), I32)
    inp("w_kj", (c.H, c.H)); inp("b_kj", (c.H, 1))
    inp("w_rbf", (c.NR, c.H))
    inp("w_sel", (c.SC, c.H, c.H))
    inp("w_bil", (c.B, c.H, c.H))
    inp("w_ji", (c.H, c.H)); inp("b_ji", (c.H, 1))
    inp("w_rb0", (c.H, c.H)); inp("b_rb0", (c.H, 1))
    inp("w_rb1", (c.H, c.H)); inp("b_rb1", (c.H, 1))
    inp("w_lin", (c.H, c.H)); inp("b_lin", (c.H, 1))
    inp("w_ra00", (c.H, c.H)); inp("b_ra00", (c.H, 1))
    inp("w_ra01", (c.H, c.H)); inp("b_ra01", (c.H, 1))
    inp("w_ra10", (c.H, c.H)); inp("b_ra10", (c.H, 1))
    inp("w_ra11", (c.H, c.H)); inp("b_ra11", (c.H, 1))

    t["out_h"] = nc.dram_tensor("out_h", [c.EC, c.H], F32,
                                kind="ExternalOutput").ap()
    t["ztab"] = nc.dram_tensor("ztab", [c.T1, c.H], F32).ap()
    t["msg"] = nc.dram_tensor("msg", [c.MSG_ROWS, c.H], F32).ap()
    return t


def _load_w(nc, pool, dram_ap, shape, tag):
    sb = pool.tile(list(shape), F32, tag=tag)
    nc.sync.dma_start(out=sb[:], in_=dram_ap)
    return sb


def _pass1(ctx: ExitStack, tc: TileContext, t, cfg: Cfg):
    """Groups of 4 tiles (512 triplets): one gather, wide matmuls."""
    c = cfg
    nc = tc.nc
    W = 512
    wpool = ctx.enter_context(tc.tile_pool(name="w1", bufs=1))
    sb = ctx.enter_context(tc.tile_pool(name="sb1", bufs=3))
    ps_a = ctx.enter_context(tc.tile_pool(name="ps1a", bufs=2, space="PSUM"))
    ps_b = ctx.enter_context(tc.tile_pool(name="ps1b", bufs=1, space="PSUM"))

    ident = wpool.tile([128, 128], F32)
    make_identity(nc, ident[:])
    w_kj = _load_w(nc, wpool, t["w_kj"][:, :], (c.H, c.H), "w_kj")
    b_kj = _load_w(nc, wpool, t["b_kj"][:, :], (c.H, 1), "b_kj")
    w_rbf = _load_w(nc, wpool, t["w_rbf"][:, :], (c.NR, c.H), "w_rbf")
    w_sel = []
    for s in range(c.SC):
        w_sel.append(_load_w(nc, wpool, t["w_sel"][s, :, :], (c.H, c.H),
                             f"w_sel{s}"))

    groups_per_bucket = c.B_PAD // W
    for g in range(c.NT1 // 4):
        s = g // groups_per_bucket
        r0 = g * W

        idxt = sb.tile([128, 4], I32)
        nc.sync.dma_start(out=idxt[:], in_=t["idx1g"][g * 128:(g + 1) * 128, :])
        xg = sb.tile([128, W], F32)
        for k in range(4):
            nc.gpsimd.indirect_dma_start(
                out=xg[:, k * 128:(k + 1) * 128], out_offset=None,
                in_=t["x_full"][:, :],
                in_offset=bass.IndirectOffsetOnAxis(ap=idxt[:, k:k + 1],
                                                    axis=0),
            )
        rbft = sb.tile([c.NR, W], F32)
        nc.sync.dma_start(out=rbft[:], in_=t["rbf1T"][:, r0:r0 + W])

        xgT_ps = ps_a.tile([128, W], F32)
        for k in range(4):
            nc.tensor.transpose(out=xgT_ps[:, k * 128:(k + 1) * 128],
                                in_=xg[:, k * 128:(k + 1) * 128],
                                identity=ident[:])
        xgT = sb.tile([128, W], F32)
        nc.vector.tensor_copy(xgT[:], xgT_ps[:])

        y_ps = ps_b.tile([128, W], F32)
        nc.tensor.matmul(out=y_ps[:], lhsT=w_kj[:], rhs=xgT[:],
                         start=True, stop=True)
        rbfe_ps = ps_b.tile([128, W], F32)
        nc.tensor.matmul(out=rbfe_ps[:], lhsT=w_rbf[:], rhs=rbft[:],
                         start=True, stop=True)

        ysil = sb.tile([128, W], F32)
        nc.scalar.activation(ysil[:], y_ps[:], AF.Silu, bias=b_kj[:, :1])
        z1 = sb.tile([128, W], F32)
        nc.vector.tensor_tensor(out=z1[:], in0=ysil[:], in1=rbfe_ps[:],
                                op=OP.mult)

        zT_ps = ps_b.tile([128, W], F32)
        nc.tensor.matmul(out=zT_ps[:], lhsT=w_sel[s][:], rhs=z1[:],
                         start=True, stop=True)
        zT = sb.tile([128, W], F32)
        nc.scalar.copy(zT[:], zT_ps[:])

        z_ps = ps_a.tile([128, W], F32)
        for k in range(4):
            nc.tensor.transpose(out=z_ps[:, k * 128:(k + 1) * 128],
                                in_=zT[:, k * 128:(k + 1) * 128],
                                identity=ident[:])
        z = sb.tile([128, W], F32)
        nc.vector.tensor_copy(z[:], z_ps[:])
        nc.sync.dma_start(
            out=t["ztab"][r0:r0 + W, :].rearrange("(n p) d -> p n d", p=128),
            in_=z[:].rearrange("p (n d) -> p n d", n=4))


def _p2_consts(nc, wpool, t, cfg):
    c = cfg
    ident = wpool.tile([128, 128], F32)
    make_identity(nc, ident[:])
    iota_i = wpool.tile([128, 128], I32)
    nc.gpsimd.iota(iota_i[:], pattern=[[1, 128]], base=0, channel_multiplier=0)
    iota_f = wpool.tile([128, 128], F32)
    nc.vector.tensor_copy(iota_f[:], iota_i[:])
    w_bil = wpool.tile([128, c.B * 128], F32)
    for b in range(c.B):
        nc.sync.dma_start(out=w_bil[:, b * 128:(b + 1) * 128],
                          in_=t["w_bil"][b, :, :])
    return ident, iota_f, w_bil


def _p2_tile_body(nc, sb, ps_y, ps_m, cfg, zgT_sb, kblk, meta, iota_f, w_bil):
    """Bilinear + sbf-scaled combine + one-hot segment matmul for one tile.

    zgT_sb[:, kblk] holds z^T for this tile; meta [128, 1+B] holds
    (idx_local, sbf_t). Returns the [128,128] SBUF msg rows tile."""
    c = cfg
    ypack_ps = ps_y.tile([128, c.B * 128], F32)
    for half in range(2):
        nc.tensor.matmul(
            out=ypack_ps[:, half * 512:(half + 1) * 512],
            lhsT=zgT_sb[:, kblk],
            rhs=w_bil[:, half * 512:(half + 1) * 512],
            start=True, stop=True)

    tmp = sb.tile([128, c.B * 128], F32)
    for b in range(c.B):
        blk = slice(b * 128, (b + 1) * 128)
        if b < 4:
            nc.scalar.activation(tmp[:, blk], ypack_ps[:, blk], AF.Copy,
                                 scale=meta[:, 1 + b:2 + b])
        else:
            nc.vector.tensor_scalar(
                out=tmp[:, blk], in0=ypack_ps[:, blk],
                scalar1=meta[:, 1 + b:2 + b], scalar2=None, op0=OP.mult)

    smat = sb.tile([128, 128], F32)
    nc.vector.tensor_scalar(out=smat[:], in0=iota_f[:],
                            scalar1=meta[:, 0:1], scalar2=None,
                            op0=OP.is_equal)

    msg_ps = ps_m.tile([128, 128], F32)
    for b in range(c.B):
        nc.tensor.matmul(out=msg_ps[:], lhsT=smat[:],
                         rhs=tmp[:, b * 128:(b + 1) * 128],
                         start=(b == 0), stop=(b == c.B - 1))
    msg_sb = sb.tile([128, 128], F32)
    nc.scalar.copy(msg_sb[:], msg_ps[:])
    return msg_sb


def _pass2(ctx: ExitStack, tc: TileContext, t, cfg: Cfg):
    """Main segment pass: groups of 2 tiles; each tile covers a fixed
    EPT-edge window and writes msg rows contiguously (no scatter)."""
    c = cfg
    nc = tc.nc
    wpool = ctx.enter_context(tc.tile_pool(name="w2", bufs=1))
    sb = ctx.enter_context(tc.tile_pool(name="sb2", bufs=3))
    ps_t = ctx.enter_context(tc.tile_pool(name="ps2t", bufs=2, space="PSUM"))
    ps_y = ctx.enter_context(tc.tile_pool(name="ps2y", bufs=2, space="PSUM"))
    ps_m = ctx.enter_context(tc.tile_pool(name="ps2m", bufs=2, space="PSUM"))

    ident, iota_f, w_bil = _p2_consts(nc, wpool, t, cfg)

    for g in range(c.NT2 // 2):
        r0 = g * 256
        idxt = sb.tile([128, 2], I32)
        nc.sync.dma_start(out=idxt[:],
                          in_=t["perm2g"][g * 128:(g + 1) * 128, :])
        zg = sb.tile([128, 256], F32)
        for k in range(2):
            nc.gpsimd.indirect_dma_start(
                out=zg[:, k * 128:(k + 1) * 128], out_offset=None,
                in_=t["ztab"][:, :],
                in_offset=bass.IndirectOffsetOnAxis(ap=idxt[:, k:k + 1],
                                                    axis=0),
            )
        meta = sb.tile([128, 2, 1 + c.B], F32)
        nc.sync.dma_start(
            out=meta[:],
            in_=t["meta2"][r0:r0 + 256, :]
            .rearrange("(n p) d -> p n d", p=128))

        zgT_ps = ps_t.tile([128, 256], F32)
        for k in range(2):
            nc.tensor.transpose(out=zgT_ps[:, k * 128:(k + 1) * 128],
                                in_=zg[:, k * 128:(k + 1) * 128],
                                identity=ident[:])
        zgT = sb.tile([128, 256], F32)
        nc.vector.tensor_copy(zgT[:], zgT_ps[:])

        for k in range(2):
            i = 2 * g + k
            msg_sb = _p2_tile_body(
                nc, sb, ps_y, ps_m, c, zgT,
                slice(k * 128, (k + 1) * 128), meta[:, k, :], iota_f, w_bil)
            nc.sync.dma_start(
                out=t["msg"][i * c.EPT:(i + 1) * c.EPT, :],
                in_=msg_sb[:c.EPT, :])


def _pass2_spill(ctx: ExitStack, tc: TileContext, t, cfg: Cfg):
    """Spill tiles: classic rank-based tiles, scatter-write by edge target.

    Runs after the main pass (its writes overwrite the zero rows the main
    pass produced for spilled edges)."""
    c = cfg
    nc = tc.nc
    wpool = ctx.enter_context(tc.tile_pool(name="w2s", bufs=1))
    sb = ctx.enter_context(tc.tile_pool(name="sb2s", bufs=3))
    ps_t = ctx.enter_context(tc.tile_pool(name="ps2st", bufs=2, space="PSUM"))
    ps_y = ctx.enter_context(tc.tile_pool(name="ps2sy", bufs=2, space="PSUM"))
    ps_m = ctx.enter_context(tc.tile_pool(name="ps2sm", bufs=2, space="PSUM"))

    ident, iota_f, w_bil = _p2_consts(nc, wpool, t, cfg)

    for i in range(c.NT2S):
        r0 = i * 128
        idxt = sb.tile([128, 1], I32)
        nc.sync.dma_start(out=idxt[:], in_=t["sperm"][r0:r0 + 128, :])
        zg = sb.tile([128, 128], F32)
        nc.gpsimd.indirect_dma_start(
            out=zg[:], out_offset=None,
            in_=t["ztab"][:, :],
            in_offset=bass.IndirectOffsetOnAxis(ap=idxt[:, :1], axis=0),
        )
        meta = sb.tile([128, 1 + c.B], F32)
        nc.sync.dma_start(out=meta[:], in_=t["smeta"][r0:r0 + 128, :])
        etg = sb.tile([128, 1], I32)
        nc.sync.dma_start(out=etg[:], in_=t["setgt"][r0:r0 + 128, :])

        zgT_ps = ps_t.tile([128, 128], F32)
        nc.tensor.transpose(out=zgT_ps[:], in_=zg[:], identity=ident[:])
        zgT = sb.tile([128, 128], F32)
        nc.vector.tensor_copy(zgT[:], zgT_ps[:])

        msg_sb = _p2_tile_body(nc, sb, ps_y, ps_m, c, zgT,
                               slice(0, 128), meta[:, :], iota_f, w_bil)
        nc.gpsimd.indirect_dma_start(
            out=t["msg"][:, :],
            out_offset=bass.IndirectOffsetOnAxis(ap=etg[:, :1], axis=0),
            in_=msg_sb[:], in_offset=None,
        )


def _edge_pass(ctx: ExitStack, tc: TileContext, t, cfg: Cfg):
    c = cfg
    nc = tc.nc
    ch = c.ECHUNK
    nblk = 4
    blk = ch // nblk
    assert blk * nblk == ch and blk <= 128

    wpool = ctx.enter_context(tc.tile_pool(name="w3", bufs=1))
    sb = ctx.enter_context(tc.tile_pool(name="sb3", bufs=3))
    ps_x = ctx.enter_context(tc.tile_pool(name="ps3x", bufs=2, space="PSUM"))
    ps_g = ctx.enter_context(tc.tile_pool(name="ps3g", bufs=2, space="PSUM"))
    ps_o = ctx.enter_context(tc.tile_pool(name="ps3o", bufs=2, space="PSUM"))

    ident = wpool.tile([128, 128], F32)
    make_identity(nc, ident[:])
    names = ["w_ji", "w_rb0", "w_rb1", "w_lin", "w_ra00", "w_ra01",
             "w_ra10", "w_ra11"]
    ws = {n: _load_w(nc, wpool, t[n][:, :], (c.H, c.H), n) for n in names}
    bs = {n: _load_w(nc, wpool, t["b" + n[1:]][:, :], (c.H, 1), "b" + n[1:])
          for n in names}

    def mm_silu(w_name, rhs_sb):
        mm_ps = ps_g.tile([128, ch], F32)
        nc.tensor.matmul(out=mm_ps[:], lhsT=ws[w_name][:], rhs=rhs_sb[:],
                         start=True, stop=True)
        out = sb.tile([128, ch], F32)
        nc.scalar.activation(out[:], mm_ps[:], AF.Silu,
                             bias=bs[w_name][:, :1])
        return out

    for ci in range(c.NCHUNK):
        e0 = ci * ch

        xr = sb.tile([blk, nblk, 128], F32)
        nc.sync.dma_start(
            out=xr[:],
            in_=t["x_own"][e0:e0 + ch, :].rearrange("(n p) d -> p n d", p=blk))
        mr = sb.tile([blk, nblk, 128], F32)
        nc.sync.dma_start(
            out=mr[:],
            in_=t["msg"][e0:e0 + ch, :].rearrange("(n p) d -> p n d", p=blk))

        xT_ps = ps_x.tile([128, ch], F32)
        mT_ps = ps_x.tile([128, ch], F32)
        for k in range(nblk):
            ksl = slice(k * blk, (k + 1) * blk)
            nc.tensor.transpose(out=xT_ps[:, ksl], in_=xr[:, k, :],
                                identity=ident[:blk, :blk])
            nc.tensor.transpose(out=mT_ps[:, ksl], in_=mr[:, k, :],
                                identity=ident[:blk, :blk])
        xT = sb.tile([128, ch], F32)
        nc.vector.tensor_copy(xT[:], xT_ps[:])
        mT = sb.tile([128, ch], F32)
        nc.scalar.copy(mT[:], mT_ps[:])

        xji = mm_silu("w_ji", xT)
        h = sb.tile([128, ch], F32)
        nc.vector.tensor_tensor(out=h[:], in0=xji[:], in1=mT[:], op=OP.add)

        # residual layer before skip
        u = mm_silu("w_rb0", h)
        v = mm_silu("w_rb1", u)
        h2 = sb.tile([128, ch], F32)
        nc.vector.tensor_tensor(out=h2[:], in0=h[:], in1=v[:], op=OP.add)

        # skip connection
        l = mm_silu("w_lin", h2)
        h3 = sb.tile([128, ch], F32)
        nc.vector.tensor_tensor(out=h3[:], in0=l[:], in1=xT[:], op=OP.add)

        # residual layers after skip
        u2 = mm_silu("w_ra00", h3)
        v2 = mm_silu("w_ra01", u2)
        h4 = sb.tile([128, ch], F32)
        nc.vector.tensor_tensor(out=h4[:], in0=h3[:], in1=v2[:], op=OP.add)

        u3 = mm_silu("w_ra10", h4)
        v3 = mm_silu("w_ra11", u3)
        h5 = sb.tile([128, ch], F32)
        nc.vector.tensor_tensor(out=h5[:], in0=h4[:], in1=v3[:], op=OP.add)

        oT_ps = ps_o.tile([blk, nblk, 128], F32)
        for k in range(nblk):
            nc.tensor.transpose(out=oT_ps[:, k, :],
                                in_=h5[:, k * blk:(k + 1) * blk],
                                identity=ident[:])
        orow = sb.tile([blk, nblk, 128], F32)
        nc.vector.tensor_copy(orow[:], oT_ps[:])
        nc.sync.dma_start(
            out=t["out_h"][e0:e0 + ch, :].rearrange("(n p) d -> p n d", p=blk),
            in_=orow[:])


def build_program(cfg: Cfg):
    nc = bacc.Bacc("TRN2", target_bir_lowering=False, debug=False,
                   enable_asserts=False, num_devices=cfg.NC)
    t = _decl_inputs(nc, cfg)
    with TileContext(nc) as tc:
        with ExitStack() as ctx:
            _pass1(ctx, tc, t, cfg)
    with TileContext(nc) as tc:
        with ExitStack() as ctx:
            _pass2(ctx, tc, t, cfg)
    with TileContext(nc) as tc:
        with ExitStack() as ctx:
            _pass2_spill(ctx, tc, t, cfg)
    with TileContext(nc) as tc:
        with ExitStack() as ctx:
            _edge_pass(ctx, tc, t, cfg)
    nc.compile()
    return nc


_PROGRAM_CACHE = {}


def get_program(cfg: Cfg):
    key = cfg
    if key not in _PROGRAM_CACHE:
        _PROGRAM_CACHE[key] = build_program(cfg)
    return _PROGRAM_CACHE[key]


def kernel(**inputs):
    cfg = kernel.cfg or FULL
    in_maps = host_prep(inputs, cfg)
    nc = get_program(cfg)
    res = run_bass_kernel_spmd(nc, in_maps, list(range(cfg.NC)))
    kernel.last_results = res
    outs = [res.results[m]["out_h"] for m in range(cfg.NC)]
    return np.concatenate(outs, axis=0)


kernel.cfg = None
kernel.last_results = None
